# revision 2
# baseline (speedup 1.0000x reference)
"""Trainium2 Bass kernel for GQA attention (B=2, L=2048, D=3072, H=24, KV=8,
HD=128, causal, half-split RoPE).

Sharding: TP=4 over heads x DP=2 over batch on 8 NeuronCores.
Core c = 4*b + s handles batch b with q-heads 6s..6s+5 and kv-heads 2s,2s+1.
Each core computes a partial o_proj output [L, D]; the host sums the 4 TP
partials per batch (the "all-reduce after o_proj" done on host at gather time).

Per-core device computation (all matmuls bf16 with fp32 PSUM accumulation):
  xT[D,L] (host-pretransposed, bf16)
  Q^T = Wq_s^T x^T  (per head [128,L]) -> RoPE -> qT
  K^T likewise per kv head -> RoPE
  V   = x Wv_s   natural layout [L, 256]
  per head, per 512-wide q-block: S^T[k,q] chunks via PE, exp on ScalarE
  (scale folded into exp), causal mask on diagonal chunks, AV and
  ones-matmul denominators accumulated in PSUM, normalize into O^T bf16,
  then o_proj partial = O^T.T @ Wo_s -> [L, D] fp32.
"""

import math

import numpy as np
import ml_dtypes

import concourse.bass as bass
import concourse.mybir as mybir
import concourse.tile as tile
from concourse import bacc
from concourse.bass_utils import run_bass_kernel_spmd

BF16NP = ml_dtypes.bfloat16

B, L, D = 2, 2048, 3072
H, KV, HD = 24, 8, 128
GROUP = H // KV          # 3
THETA = 500000.0
SCALE = HD ** -0.5
N_CORES = 8
TP = 4                   # tensor-parallel over heads
NQH = H // TP            # 6 q heads per core
NKH = KV // TP           # 2 kv heads per core
QCOLS = NQH * HD         # 768
KCOLS = NKH * HD         # 256
ND = D // 128            # 24 contraction chunks
NLT = L // 128           # 16 l-tiles
NB = L // 512            # 4 q-blocks
BF = mybir.dt.bfloat16
F32 = mybir.dt.float32


def _ls(i, w=512):
    return slice(i * w, (i + 1) * w)


def _rope_tables():
    half = HD // 2
    inv_freq = 1.0 / (THETA ** (np.arange(half, dtype=np.float64) / half))
    ang = np.arange(L, dtype=np.float64)[:, None] * inv_freq[None, :]  # [L, 64]
    cosT = np.cos(ang).T.astype(np.float32)   # [64, L]
    sinT = np.sin(ang).T.astype(np.float32)
    cosF = np.concatenate([cosT, cosT], 0)    # [128, L]
    sinF = np.concatenate([-sinT, sinT], 0)   # rows 0:64 get -sin
    return cosF.astype(BF16NP), sinF.astype(BF16NP)


def _mask_tiles():
    # mask[r, m, c] = 1 if causal-allowed for diagonal chunk offset m:
    # k = 128*j + r, q = 512*b + c, m = j - 4*b; allowed iff c >= 128*m + r
    r = np.arange(128)[:, None, None]
    m = np.arange(4)[None, :, None]
    c = np.arange(512)[None, None, :]
    return (c >= 128 * m + r).astype(BF16NP)  # [128, 4, 512]


def _emit(nc):
    xT = nc.dram_tensor("xT", [D, L], BF, kind="ExternalInput")
    wq = nc.dram_tensor("wq", [D, QCOLS], BF, kind="ExternalInput")
    wk = nc.dram_tensor("wk", [D, KCOLS], BF, kind="ExternalInput")
    wv = nc.dram_tensor("wv", [D, KCOLS], BF, kind="ExternalInput")
    wo = nc.dram_tensor("wo", [QCOLS, D], BF, kind="ExternalInput")
    out = nc.dram_tensor("out", [L, D], F32, kind="ExternalOutput")

    cosF, sinF = _rope_tables()
    cosc = nc.inline_tensor(np.ascontiguousarray(cosF), name="cosc")
    sinc = nc.inline_tensor(np.ascontiguousarray(sinF), name="sinc")
    maskc = nc.inline_tensor(np.ascontiguousarray(_mask_tiles()), name="maskc")

    Exp = mybir.ActivationFunctionType.Exp

    with tile.TileContext(nc) as tc:
        with (
            tc.tile_pool(name="persist", bufs=1) as P,
        ):
            cos_sb = P.tile([128, L], BF, tag="cos")
            nc.sync.dma_start(out=cos_sb, in_=cosc.ap())
            sin_sb = P.tile([128, L], BF, tag="sin")
            nc.sync.dma_start(out=sin_sb, in_=sinc.ap())
            mask_sb = P.tile([128, 4, 512], BF, tag="mask")
            nc.sync.dma_start(out=mask_sb, in_=maskc.ap())
            ones_sb = P.tile([128, 128], BF, tag="ones")
            nc.vector.memset(ones_sb, 1.0)

            # persistent activations
            qkT = [
                P.tile([128, L], BF, tag=f"qkT{i}", name=f"qkT{i}")
                for i in range(NQH + NKH)
            ]
            v_sb = P.tile([128, NLT, KCOLS], BF, tag="vsb")
            oT_sb = P.tile([128, NQH, L], BF, tag="oT")

            # ---- phase 1: projections + rope, processed in two L halves ----
            with (
                tc.tile_pool(name="xt", bufs=1) as XT,
                tc.tile_pool(name="wstream", bufs=2) as WS,
                tc.tile_pool(name="ropet", bufs=4) as RT,
                tc.tile_pool(name="ps_qk", bufs=4, space="PSUM") as PQ,
                tc.tile_pool(name="ps_v", bufs=2, space="PSUM") as PV,
            ):
                wv_sb = XT.tile([128, ND, KCOLS], BF, tag="wvsb")
                wv_r = wv.ap().rearrange("(dc p) n -> p dc n", p=128)
                for d in range(ND):
                    nc.sync.dma_start(out=wv_sb[:, d, :], in_=wv_r[:, d, :])

                xT_r = xT.ap().rearrange("(dc p) l -> p dc l", p=128)
                wq_r = wq.ap().rearrange("(dc p) (mt m) -> p dc mt m", p=128, m=128)
                wk_r = wk.ap().rearrange("(dc p) (mt m) -> p dc mt m", p=128, m=128)

                LH = L // 2  # 1024 per half
                for half in range(2):
                    hs = half * LH
                    xt_sb = XT.tile([128, ND, LH], BF, tag="xt")
                    for d in range(ND):
                        nc.sync.dma_start(
                            out=xt_sb[:, d, :], in_=xT_r[:, d, hs:hs + LH]
                        )
                    # Q^T / K^T projections + rope
                    for mi in range(NQH + NKH):
                        wm = WS.tile([128, ND, 128], BF, tag="wm")
                        src = wq_r[:, :, mi, :] if mi < NQH else wk_r[:, :, mi - NQH, :]
                        nc.sync.dma_start(out=wm, in_=src)
                        for lb in range(LH // 512):
                            gcol = hs + lb * 512  # global l offset
                            ps = PQ.tile([128, 512], F32, tag="psqk")
                            for d in range(ND):
                                nc.tensor.matmul(
                                    ps,
                                    lhsT=wm[:, d, :],
                                    rhs=xt_sb[:, d, _ls(lb)],
                                    start=(d == 0),
                                    stop=(d == ND - 1),
                                )
                            qkb = RT.tile([128, 512], BF, tag="qkb")
                            nc.scalar.copy(qkb, ps)
                            rot = RT.tile([128, 512], BF, tag="rot")
                            nc.vector.tensor_copy(out=rot[0:64, :], in_=qkb[64:128, :])
                            nc.vector.tensor_copy(out=rot[64:128, :], in_=qkb[0:64, :])
                            t1 = RT.tile([128, 512], BF, tag="t1")
                            nc.vector.tensor_mul(
                                t1, qkb, cos_sb[:, gcol:gcol + 512]
                            )
                            t2 = RT.tile([128, 512], BF, tag="t2")
                            nc.vector.tensor_mul(
                                t2, rot, sin_sb[:, gcol:gcol + 512]
                            )
                            nc.vector.tensor_add(
                                qkT[mi][:, gcol:gcol + 512], t1, t2
                            )
                    # V projection (natural layout)
                    for lt in range(LH // 128):
                        glt = half * (LH // 128) + lt
                        pv = PV.tile([128, KCOLS], F32, tag="psv")
                        for d in range(ND):
                            nc.tensor.matmul(
                                pv,
                                lhsT=xt_sb[:, d, lt * 128:(lt + 1) * 128],
                                rhs=wv_sb[:, d, :],
                                start=(d == 0),
                                stop=(d == ND - 1),
                            )
                        nc.scalar.copy(v_sb[:, glt, :], pv)

            # ---- phase 2: attention ----
            with (
                tc.tile_pool(name="wo", bufs=1) as WO,
                tc.tile_pool(name="p2", bufs=4) as P2,
                tc.tile_pool(name="stage", bufs=4) as SG,
            ):
                wo_sb = WO.tile([128, NQH, D], BF, tag="wosb")
                wo_r = wo.ap().rearrange("(c p) n -> p c n", p=128)
                for c in range(NQH):
                    nc.sync.dma_start(out=wo_sb[:, c, :], in_=wo_r[:, c, :])

                with (
                    tc.tile_pool(name="ps_sc", bufs=3, space="PSUM") as PS,
                    tc.tile_pool(name="ps_o", bufs=2, space="PSUM") as PO,
                    tc.tile_pool(name="ps_sum", bufs=2, space="PSUM") as PSM,
                ):
                    for h in range(NQH):
                        kv = h // GROUP
                        qT = qkT[h]
                        kT = qkT[NQH + kv]
                        for b in range(NB):
                            nch = 4 * (b + 1)
                            po = PO.tile([128, 512], F32, tag="po")
                            psm = PSM.tile([128, 512], F32, tag="psm")
                            for j in range(nch):
                                sc = PS.tile([128, 512], F32, tag="sc")
                                nc.tensor.matmul(
                                    sc,
                                    lhsT=kT[:, j * 128:(j + 1) * 128],
                                    rhs=qT[:, _ls(b)],
                                    start=True,
                                    stop=True,
                                )
                                pt = P2.tile([128, 512], BF, tag="pt")
                                nc.scalar.activation(pt, sc, Exp, scale=SCALE)
                                if j >= 4 * b:
                                    nc.vector.tensor_mul(
                                        pt, pt, mask_sb[:, j - 4 * b, :]
                                    )
                                nc.tensor.matmul(
                                    po,
                                    lhsT=v_sb[:, j, kv * 128:(kv + 1) * 128],
                                    rhs=pt,
                                    start=(j == 0),
                                    stop=(j == nch - 1),
                                )
                                nc.tensor.matmul(
                                    psm,
                                    lhsT=ones_sb,
                                    rhs=pt,
                                    start=(j == 0),
                                    stop=(j == nch - 1),
                                )
                            rc = P2.tile([128, 512], F32, tag="rc")
                            nc.vector.reciprocal(rc, psm)
                            nc.vector.tensor_mul(oT_sb[:, h, _ls(b)], po, rc)

                # ---- phase 3: o_proj partial ----
                with (
                    tc.tile_pool(name="ps_op", bufs=4, space="PSUM") as POP,
                ):
                    out_r = out.ap().rearrange(
                        "(lt p) (et n) -> p lt et n", p=128, n=512
                    )
                    for lt in range(NLT):
                        for e in range(D // 512):
                            pp = POP.tile([128, 512], F32, tag="pp")
                            for c in range(NQH):
                                nc.tensor.matmul(
                                    pp,
                                    lhsT=oT_sb[:, c, lt * 128:(lt + 1) * 128],
                                    rhs=wo_sb[:, c, _ls(e)],
                                    start=(c == 0),
                                    stop=(c == NQH - 1),
                                )
                            st = SG.tile([128, 512], F32, tag="st")
                            if e % 2 == 0:
                                nc.vector.tensor_copy(st, pp)
                            else:
                                nc.scalar.copy(st, pp)
                            nc.sync.dma_start(out=out_r[:, lt, e, :], in_=st)
    return nc


_NC_CACHE = {}


def build():
    if "nc" not in _NC_CACHE:
        nc = bacc.Bacc(
            "TRN2", target_bir_lowering=False, debug=False, num_devices=N_CORES
        )
        _emit(nc)
        nc.compile()
        _NC_CACHE["nc"] = nc
    return _NC_CACHE["nc"]


def prep_in_maps(x, Wq, Wk, Wv, Wo):
    """Shard + cast + layout the full inputs into 8 per-core input maps."""
    x = np.asarray(x)
    Wq, Wk, Wv, Wo = (np.asarray(a) for a in (Wq, Wk, Wv, Wo))
    in_maps = []
    wq_s = [np.ascontiguousarray(Wq[:, s * QCOLS:(s + 1) * QCOLS]).astype(BF16NP)
            for s in range(TP)]
    wk_s = [np.ascontiguousarray(Wk[:, s * KCOLS:(s + 1) * KCOLS]).astype(BF16NP)
            for s in range(TP)]
    wv_s = [np.ascontiguousarray(Wv[:, s * KCOLS:(s + 1) * KCOLS]).astype(BF16NP)
            for s in range(TP)]
    wo_s = [np.ascontiguousarray(Wo[s * QCOLS:(s + 1) * QCOLS, :]).astype(BF16NP)
            for s in range(TP)]
    xT_b = [np.ascontiguousarray(x[b].T).astype(BF16NP) for b in range(B)]
    for core in range(N_CORES):
        b, s = divmod(core, TP)
        in_maps.append({
            "xT": xT_b[b],
            "wq": wq_s[s],
            "wk": wk_s[s],
            "wv": wv_s[s],
            "wo": wo_s[s],
        })
    return in_maps


def kernel(x, Wq, Wk, Wv, Wo):
    nc = build()
    in_maps = prep_in_maps(x, Wq, Wk, Wv, Wo)
    res = run_bass_kernel_spmd(nc, in_maps, list(range(N_CORES)))
    out = np.zeros((B, L, D), np.float32)
    for core in range(N_CORES):
        b, _s = divmod(core, TP)
        out[b] += res.results[core]["out"]
    return out


# revision 3
# speedup vs baseline: 2.8731x; 2.8731x over previous
"""Trainium2 Bass kernel for GQA attention (B=2, L=2048, D=3072, H=24, KV=8,
HD=128, causal, half-split RoPE).

Sharding: TP=4 over heads x DP=2 over batch on 8 NeuronCores.
Core c = 4*b + s handles batch b with q-heads 6s..6s+5 and kv-heads 2s,2s+1.
Each core computes a partial o_proj output [L, D]; the host sums the 4 TP
partials per batch (the "all-reduce after o_proj" done on host at gather time).

Per-core device computation (all matmuls bf16 with fp32 PSUM accumulation):
  xT[D,L] (host-pretransposed, bf16)
  Q^T = Wq_s^T x^T  (per head [128,L]) -> RoPE -> qT
  K^T likewise per kv head -> RoPE
  V   = x Wv_s   natural layout [L, 256]
  per head, per 512-wide q-block: S^T[k,q] chunks via PE, exp on ScalarE
  (scale folded into exp), causal mask on diagonal chunks, AV and
  ones-matmul denominators accumulated in PSUM, normalize into O^T bf16,
  then o_proj partial = O^T.T @ Wo_s -> [L, D] fp32.
"""

import math

import numpy as np
import ml_dtypes

import concourse.bass as bass
import concourse.mybir as mybir
import concourse.tile as tile
from concourse import bacc
from concourse.bass_utils import run_bass_kernel_spmd

BF16NP = ml_dtypes.bfloat16

B, L, D = 2, 2048, 3072
H, KV, HD = 24, 8, 128
GROUP = H // KV          # 3
THETA = 500000.0
SCALE = HD ** -0.5
N_CORES = 8
TP = 4                   # tensor-parallel over heads
NQH = H // TP            # 6 q heads per core
NKH = KV // TP           # 2 kv heads per core
QCOLS = NQH * HD         # 768
KCOLS = NKH * HD         # 256
ND = D // 128            # 24 contraction chunks
NLT = L // 128           # 16 l-tiles
NB = L // 512            # 4 q-blocks
BF = mybir.dt.bfloat16
F32 = mybir.dt.float32


def _ls(i, w=512):
    return slice(i * w, (i + 1) * w)


def _rope_tables():
    half = HD // 2
    inv_freq = 1.0 / (THETA ** (np.arange(half, dtype=np.float64) / half))
    ang = np.arange(L, dtype=np.float64)[:, None] * inv_freq[None, :]  # [L, 64]
    cosT = np.cos(ang).T.astype(np.float32)   # [64, L]
    sinT = np.sin(ang).T.astype(np.float32)
    cosF = np.concatenate([cosT, cosT], 0)    # [128, L]
    sinF = np.concatenate([-sinT, sinT], 0)   # rows 0:64 get -sin
    return cosF.astype(BF16NP), sinF.astype(BF16NP)


def _mask_tiles():
    # mask[r, m, c] = 1 if causal-allowed for diagonal chunk offset m:
    # k = 128*j + r, q = 512*b + c, m = j - 4*b; allowed iff c >= 128*m + r
    r = np.arange(128)[:, None, None]
    m = np.arange(4)[None, :, None]
    c = np.arange(512)[None, None, :]
    return (c >= 128 * m + r).astype(BF16NP)  # [128, 4, 512]


def _emit(nc):
    xT = nc.dram_tensor("xT", [D, L], BF, kind="ExternalInput")
    wq = nc.dram_tensor("wq", [D, QCOLS], BF, kind="ExternalInput")
    wk = nc.dram_tensor("wk", [D, KCOLS], BF, kind="ExternalInput")
    wv = nc.dram_tensor("wv", [D, KCOLS], BF, kind="ExternalInput")
    wo = nc.dram_tensor("wo", [QCOLS, D], BF, kind="ExternalInput")
    out = nc.dram_tensor("out", [L, D], BF, kind="ExternalOutput")

    cosF, sinF = _rope_tables()
    cosc = nc.inline_tensor(np.ascontiguousarray(cosF), name="cosc")
    sinc = nc.inline_tensor(np.ascontiguousarray(sinF), name="sinc")
    maskc = nc.inline_tensor(np.ascontiguousarray(_mask_tiles()), name="maskc")

    Exp = mybir.ActivationFunctionType.Exp

    with tile.TileContext(nc) as tc:
        with (
            tc.tile_pool(name="persist", bufs=1) as P,
        ):
            cos_sb = P.tile([128, L], BF, tag="cos")
            nc.sync.dma_start(out=cos_sb, in_=cosc.ap())
            sin_sb = P.tile([128, L], BF, tag="sin")
            nc.sync.dma_start(out=sin_sb, in_=sinc.ap())
            mask_sb = P.tile([128, 4, 512], BF, tag="mask")
            nc.sync.dma_start(out=mask_sb, in_=maskc.ap())
            ones_sb = P.tile([128, 128], BF, tag="ones")
            nc.vector.memset(ones_sb, 1.0)

            # persistent activations
            qkT = [
                P.tile([128, L], BF, tag=f"qkT{i}", name=f"qkT{i}")
                for i in range(NQH + NKH)
            ]
            v_sb = P.tile([128, NLT, KCOLS], BF, tag="vsb")
            oT_sb = P.tile([128, NQH, L], BF, tag="oT")

            # ---- phase 1: projections + rope, processed in two L halves ----
            with (
                tc.tile_pool(name="xt", bufs=1) as XT,
                tc.tile_pool(name="wstream", bufs=2) as WS,
                tc.tile_pool(name="ropet", bufs=4) as RT,
                tc.tile_pool(name="ps_qk", bufs=4, space="PSUM") as PQ,
                tc.tile_pool(name="ps_v", bufs=2, space="PSUM") as PV,
            ):
                wv_sb = XT.tile([128, ND, KCOLS], BF, tag="wvsb")
                wv_r = wv.ap().rearrange("(dc p) n -> p dc n", p=128)
                for d in range(ND):
                    eng = nc.sync if d % 2 == 0 else nc.gpsimd
                    eng.dma_start(out=wv_sb[:, d, :], in_=wv_r[:, d, :])

                xT_r = xT.ap().rearrange("(dc p) l -> p dc l", p=128)
                wq_r = wq.ap().rearrange("(dc p) (mt m) -> p dc mt m", p=128, m=128)
                wk_r = wk.ap().rearrange("(dc p) (mt m) -> p dc mt m", p=128, m=128)

                LH = L // 2  # 1024 per half
                for half in range(2):
                    hs = half * LH
                    xt_sb = XT.tile([128, ND, LH], BF, tag="xt")
                    for d in range(ND):
                        eng = nc.sync if d % 2 == 0 else nc.gpsimd
                        eng.dma_start(
                            out=xt_sb[:, d, :], in_=xT_r[:, d, hs:hs + LH]
                        )
                    # Q^T / K^T projections + rope
                    for mi in range(NQH + NKH):
                        wm = WS.tile([128, ND, 128], BF, tag="wm")
                        src = wq_r[:, :, mi, :] if mi < NQH else wk_r[:, :, mi - NQH, :]
                        nc.gpsimd.dma_start(out=wm, in_=src)
                        for lb in range(LH // 512):
                            gcol = hs + lb * 512  # global l offset
                            ps = PQ.tile([128, 512], F32, tag="psqk")
                            for d in range(ND):
                                nc.tensor.matmul(
                                    ps,
                                    lhsT=wm[:, d, :],
                                    rhs=xt_sb[:, d, _ls(lb)],
                                    start=(d == 0),
                                    stop=(d == ND - 1),
                                )
                            qkb = RT.tile([128, 512], BF, tag="qkb")
                            nc.scalar.copy(qkb, ps)
                            rot = RT.tile([128, 512], BF, tag="rot")
                            nc.vector.tensor_copy(out=rot[0:64, :], in_=qkb[64:128, :])
                            nc.vector.tensor_copy(out=rot[64:128, :], in_=qkb[0:64, :])
                            t1 = RT.tile([128, 512], BF, tag="t1")
                            nc.vector.tensor_mul(
                                t1, qkb, cos_sb[:, gcol:gcol + 512]
                            )
                            t2 = RT.tile([128, 512], BF, tag="t2")
                            nc.vector.tensor_mul(
                                t2, rot, sin_sb[:, gcol:gcol + 512]
                            )
                            nc.vector.tensor_add(
                                qkT[mi][:, gcol:gcol + 512], t1, t2
                            )
                    # V projection (natural layout)
                    for lt in range(LH // 128):
                        glt = half * (LH // 128) + lt
                        pv = PV.tile([128, KCOLS], F32, tag="psv")
                        for d in range(ND):
                            nc.tensor.matmul(
                                pv,
                                lhsT=xt_sb[:, d, lt * 128:(lt + 1) * 128],
                                rhs=wv_sb[:, d, :],
                                start=(d == 0),
                                stop=(d == ND - 1),
                            )
                        nc.scalar.copy(v_sb[:, glt, :], pv)

            # ---- phase 2: attention ----
            with (
                tc.tile_pool(name="wo", bufs=1) as WO,
                tc.tile_pool(name="p2", bufs=4) as P2,
                tc.tile_pool(name="stage", bufs=4) as SG,
            ):
                wo_sb = WO.tile([128, NQH, D], BF, tag="wosb")
                wo_r = wo.ap().rearrange("(c p) n -> p c n", p=128)
                for c in range(NQH):
                    eng = nc.sync if c % 2 == 0 else nc.gpsimd
                    eng.dma_start(out=wo_sb[:, c, :], in_=wo_r[:, c, :])

                with (
                    tc.tile_pool(name="ps_sc", bufs=3, space="PSUM") as PS,
                    tc.tile_pool(name="ps_o", bufs=2, space="PSUM") as PO,
                    tc.tile_pool(name="ps_sum", bufs=2, space="PSUM") as PSM,
                ):
                    for h in range(NQH):
                        kv = h // GROUP
                        qT = qkT[h]
                        kT = qkT[NQH + kv]
                        for b in range(NB):
                            nch = 4 * (b + 1)
                            po = PO.tile([128, 512], F32, tag="po")
                            psm = PSM.tile([128, 512], F32, tag="psm")
                            for j in range(nch):
                                sc = PS.tile([128, 512], F32, tag="sc")
                                nc.tensor.matmul(
                                    sc,
                                    lhsT=kT[:, j * 128:(j + 1) * 128],
                                    rhs=qT[:, _ls(b)],
                                    start=True,
                                    stop=True,
                                )
                                pt = P2.tile([128, 512], BF, tag="pt")
                                nc.scalar.activation(pt, sc, Exp, scale=SCALE)
                                if j >= 4 * b:
                                    nc.vector.tensor_mul(
                                        pt, pt, mask_sb[:, j - 4 * b, :]
                                    )
                                nc.tensor.matmul(
                                    po,
                                    lhsT=v_sb[:, j, kv * 128:(kv + 1) * 128],
                                    rhs=pt,
                                    start=(j == 0),
                                    stop=(j == nch - 1),
                                )
                                nc.tensor.matmul(
                                    psm,
                                    lhsT=ones_sb,
                                    rhs=pt,
                                    start=(j == 0),
                                    stop=(j == nch - 1),
                                )
                            rc = P2.tile([128, 512], F32, tag="rc")
                            nc.vector.reciprocal(rc, psm)
                            nc.vector.tensor_mul(oT_sb[:, h, _ls(b)], po, rc)

                # ---- phase 3: o_proj partial ----
                with (
                    tc.tile_pool(name="ps_op", bufs=4, space="PSUM") as POP,
                ):
                    out_r = out.ap().rearrange(
                        "(lt p) (et n) -> p lt et n", p=128, n=512
                    )
                    for lt in range(NLT):
                        for e in range(D // 512):
                            pp = POP.tile([128, 512], F32, tag="pp")
                            for c in range(NQH):
                                nc.tensor.matmul(
                                    pp,
                                    lhsT=oT_sb[:, c, lt * 128:(lt + 1) * 128],
                                    rhs=wo_sb[:, c, _ls(e)],
                                    start=(c == 0),
                                    stop=(c == NQH - 1),
                                )
                            st = SG.tile([128, 512], BF, tag="st")
                            if e % 2 == 0:
                                nc.vector.tensor_copy(st, pp)
                            else:
                                nc.scalar.copy(st, pp)
                            nc.scalar.dma_start(out=out_r[:, lt, e, :], in_=st)
    return nc


_NC_CACHE = {}


def build():
    if "nc" not in _NC_CACHE:
        nc = bacc.Bacc(
            "TRN2", target_bir_lowering=False, debug=False, num_devices=N_CORES
        )
        _emit(nc)
        nc.compile()
        _NC_CACHE["nc"] = nc
    return _NC_CACHE["nc"]


def prep_in_maps(x, Wq, Wk, Wv, Wo):
    """Shard + cast + layout the full inputs into 8 per-core input maps."""
    x = np.asarray(x)
    Wq, Wk, Wv, Wo = (np.asarray(a) for a in (Wq, Wk, Wv, Wo))
    in_maps = []
    wq_s = [np.ascontiguousarray(Wq[:, s * QCOLS:(s + 1) * QCOLS]).astype(BF16NP)
            for s in range(TP)]
    wk_s = [np.ascontiguousarray(Wk[:, s * KCOLS:(s + 1) * KCOLS]).astype(BF16NP)
            for s in range(TP)]
    wv_s = [np.ascontiguousarray(Wv[:, s * KCOLS:(s + 1) * KCOLS]).astype(BF16NP)
            for s in range(TP)]
    wo_s = [np.ascontiguousarray(Wo[s * QCOLS:(s + 1) * QCOLS, :]).astype(BF16NP)
            for s in range(TP)]
    xT_b = [np.ascontiguousarray(x[b].T).astype(BF16NP) for b in range(B)]
    for core in range(N_CORES):
        b, s = divmod(core, TP)
        in_maps.append({
            "xT": xT_b[b],
            "wq": wq_s[s],
            "wk": wk_s[s],
            "wv": wv_s[s],
            "wo": wo_s[s],
        })
    return in_maps


def kernel(x, Wq, Wk, Wv, Wo):
    nc = build()
    in_maps = prep_in_maps(x, Wq, Wk, Wv, Wo)
    res = run_bass_kernel_spmd(nc, in_maps, list(range(N_CORES)))
    out = np.zeros((B, L, D), np.float32)
    for core in range(N_CORES):
        b, _s = divmod(core, TP)
        out[b] += res.results[core]["out"].astype(np.float32)
    return out
